# revision 31
# baseline (speedup 1.0000x reference)
"""Bilateral filter (K=7, guide channels=3) Trainium2 Bass kernel.

Contract: kernel(**inputs) takes FULL unsharded numpy inputs
(input [2,32,256,256] f32, input_for_kernel [2,3,256,256] f32,
sigma_for_kernel scalar f32) and returns the full output [2,32,256,256] f32.
Shards internally over 8 NeuronCores: (batch=2) x (4 h-blocks of 64 rows).

Math (identical to the reference up to fp rounding; the spatial-gaussian
normalization cancels in ker/norm):
  m_u[p]   = exp(-0.5*sum_c (g[c,p+u]-g[c,p])^2) * exp(-0.5*(uy^2+ux^2)/s^2)
  out[c,p] = sum_u m_u[p]*in[c,p+u] / sum_u m_u[p]        (zero padding)

Symmetry m_{-u}[p] = m_u[p-u] -> only 25 maps computed on an extended grid,
all 49 aligned maps then materialized by DMA remap (DMA can cross partitions;
compute engines cannot, so every row shift lives in a free dim).
"""

import numpy as np

B, C, H, W = 2, 32, 256, 256
CG = 3
R = 3                      # K//2
NB = 4                     # h-blocks per batch
RB = H // NB               # 64 out rows per core
NCORES = 8

GR = RB + 2 * R            # 70 rows   (out rows -3..66)
GX = W + 4 * R             # 268 guide cols (-6..261)
IX = W + 2 * R             # 262 input cols (-3..258)
MR = RB + R                # 67 map rows (-3..63)
MJ = W + 2 * R             # 262 map cols (-3..258)
MS = W + 2 * R             # 262 per-map stride in K25 (even)
WH = 2                     # w halves in apply layout
XC = W // WH               # 128
XW = XC + 2 * R            # 134 per-half x window

UPLUS = [(0, ux) for ux in range(0, R + 1)] + [
    (uy, ux) for uy in range(1, R + 1) for ux in range(-R, R + 1)
]
ALL_U = []
for uy in range(-R, R + 1):
    for ux in range(-R, R + 1):
        if (uy, ux) in UPLUS:
            ALL_U.append((uy, ux, UPLUS.index((uy, ux)), False))
        else:
            ALL_U.append((uy, ux, UPLUS.index((-uy, -ux)), True))

_COMPILED = None


def _build_nc(legalize=True):
    import concourse.bass as bass
    import concourse.mybir as mybir
    from concourse.bass import AP
    from concourse.tile import TileContext

    fp32 = mybir.dt.float32
    fp16 = mybir.dt.float16
    ALU = mybir.AluOpType
    ACTF = mybir.ActivationFunctionType

    nc = bass.Bass()

    guide_d = nc.declare_dram_parameter("guide", [CG, GR, GX], fp32, isOutput=False)
    inp_d = nc.declare_dram_parameter("inp", [C, GR, IX], fp32, isOutput=False)
    rr_d = nc.declare_dram_parameter("rr25", [1, 25], fp32, isOutput=False)
    sig_d = nc.declare_dram_parameter("sigma", [1, 1], fp32, isOutput=False)
    ident_d = nc.declare_dram_parameter("ident", [128, 128], fp16, isOutput=False)
    out_d = nc.declare_dram_parameter("out", [C, RB, W], fp32, isOutput=True)

    def sb(t, p0, pn, off, dims):
        """AP into sbuf/psum tensor: partitions [p0, p0+pn), free offset+dims."""
        sl = t[p0:p0 + pn]
        return AP(sl.tensor, sl.offset + off, [sl.ap[0], *dims])

    def dr_ap(d, off, dims):
        full = d[:]
        return AP(full.tensor, full.offset + off, dims)

    from contextlib import ExitStack

    with TileContext(nc) as tc, ExitStack() as es:
        def mk(name, shape, dt):
            return es.enter_context(nc.sbuf_tensor(name, shape, dt))

        # aliasing region: guide-phase scratch (G4/D3B/SREDG/INH) is dead by
        # the time INB7O (odd-parity input copy) is written; OverlapTracker
        # fences reads->writes by byte range.
        _base0 = ((nc.sbuf_base + 31) // 32) * 32
        _ARENA_BYTES = 190 * 1024
        es.enter_context(nc.sbuf_tensor("ARENA", [128, _ARENA_BYTES], mybir.dt.uint8))
        _off = [_base0]
        def at(name, shape, dt, offset=None):
            if offset is None:
                offset = _off[0]
            import functools, operator
            sz = functools.reduce(operator.mul, shape[1:]) * mybir.dt.size(dt)
            h = nc.alloc_sbuf_tensor_at(name, shape, dt, offset=offset, align_bytes=32)
            _off[0] = max(_off[0], offset + ((sz + 31) // 32) * 32)
            return h

        INB7 = at("INB7", [128, C * 7 * XW], fp16)          # (c,dr,x)
        _arena = _off[0]
        INB7O = at("INB7O", [128, C * 7 * XW], fp16, _arena)
        # guide-phase scratch aliases INB7O's bytes (dead before INB7O written)
        def _al(n):
            return ((n + 31) // 32) * 32
        NSLOT = 6
        _g4b = _al(4 * CG * GX * 4)
        _d3b = _al(NSLOT * CG * MJ * 4)
        _sqb = _al(NSLOT * CG * MJ * 4)
        _srb = _al(NSLOT * MJ * 4)
        G4 = at("G4", [128, 4 * CG * GX], fp32, _arena)
        D3R = at("D3R", [128, NSLOT * CG * MJ], fp32, _arena + _g4b)
        SQR = at("SQR", [128, NSLOT * CG * MJ], fp32, _arena + _g4b + _d3b)
        SRR = at("SRR", [128, NSLOT * MJ], fp32, _arena + _g4b + _d3b + _sqb)
        assert _g4b + _d3b + _sqb + _srb <= C * 7 * XW * 2, (
            _g4b + _d3b + _sqb + _srb, C * 7 * XW * 2)
        _off[0] = _arena + ((C * 7 * XW * 2 + 31) // 32) * 32
        K25 = at("K25", [128, 25 * MS], fp16)               # (m,j)
        KA = at("KA", [128, 49 * XC], fp16)                 # (u,x)
        NORM = at("NORM", [128, XC], fp32)
        RCP = at("RCP", [128, XC], fp32)
        BROWT = at("BROWT", [128, 25], fp32)
        IDENT = at("IDENT", [128, 128], fp16)
        PBUFS = [at(f"P{i}T", [128, 4096], fp16) for i in range(4)]
        OUTC = at("OUTC", [128, 2 * 8 * XC], fp32)
        BROW = at("BROW", [1, 25], fp32)
        SIG = at("SIG", [1, 1], fp32)
        SIG2 = at("SIG2", [1, 1], fp32)
        IS2 = at("IS2", [1, 1], fp32)
        RR = at("RR", [1, 25], fp32)
        assert _off[0] <= _base0 + _ARENA_BYTES, (_off[0], _base0)
        ACC = es.enter_context(nc.psum_tensor("ACC", [128, 4096], fp32))
        if True:
            v, s, g, t, sync = nc.vector, nc.scalar, nc.gpsimd, nc.tensor, nc.sync

            # ---- guide load first (gates the first subs) ----
            for dy in range(4):
                dst = sb(G4, 0, MR, dy * (CG * GX), [[GX, CG], [1, GX]])
                src = dr_ap(guide_d, dy * GX, [[GX, MR], [GR * GX, CG], [1, GX]])
                sync.dma_start(out=dst, in_=src)

            # ---- constants ----
            sync.dma_start(out=RR[:], in_=rr_d[:])
            sync.dma_start(out=SIG[:], in_=sig_d[:])
            sync.dma_start(out=IDENT[:], in_=ident_d[:])
            v.tensor_tensor(out=SIG2[:], in0=SIG[:], in1=SIG[:], op=ALU.mult)
            v.reciprocal(out=IS2[:], in_=SIG2[:])
            v.tensor_scalar(out=BROW[:], in0=RR[:], scalar1=IS2[0:1, 0:1],
                            scalar2=-0.5, op0=ALU.mult, op1=ALU.mult)
            # broadcast partition 0 -> all via DRAM round-trip (0-step read)
            wscr = nc.dram_tensor("wscr", [1, 25], fp32)
            sync.dma_start(out=wscr[:], in_=BROW[:])
            sync.dma_start(out=BROWT[:],
                           in_=dr_ap(wscr, 0, [[0, 128], [1, 25]]))

            # ---- stage fp16 input in DRAM (software-DGE cast DMA), then
            #      row-replicated loads straight from DRAM (overlapping reads).
            #      1 leading pad element so the x-1 (odd-parity) reads stay
            #      in bounds. ----
            INHD = nc.dram_tensor("INHD", [1, C * GR * IX + 2], fp16)
            g.dma_start(out=dr_ap(INHD, 1, [[1, C * GR * IX]]),
                        in_=dr_ap(inp_d, 0, [[1, C * GR * IX]]))
            # dr order matches apply-map order (m=0 is uy=0 -> dr3 first)
            DR_ORDER = [3, 4, 2, 5, 1, 6, 0]
            for dr in DR_ORDER:
                for wh in range(WH):
                    dst = sb(INB7, wh * 64, 64, dr * XW, [[7 * XW, C], [1, XW]])
                    src = dr_ap(INHD, 1 + dr * IX + wh * XC,
                                [[IX, 64], [GR * IX, C], [1, XW]])
                    sync.dma_start(out=dst, in_=src)

            # kA slot order: aligned maps at slot=m, shifted at 24+m
            ka_slot = {}
            for (uy, ux, m, shifted) in ALL_U:
                ka_slot[(uy, ux)] = m if not shifted else 24 + m
            # batch tables keyed by the last map index that completes them
            AL_CHUNKS = {}
            CH = 4
            for m0c in range(0, 25, CH):
                nmap = min(CH, 25 - m0c)
                AL_CHUNKS.setdefault(m0c + nmap - 1, []).append((m0c, nmap))
            SH_GROUPS = {}
            _vy_base = {0: 1, 1: 4, 2: 11, 3: 18}
            for (vy, mb, nmap) in [(0, 1, 3), (1, 4, 4), (1, 8, 3), (2, 11, 4),
                                   (2, 15, 3), (3, 18, 4), (3, 22, 3)]:
                vxmin = ((-3 if vy >= 1 else 1) + (mb - _vy_base[vy]))
                SH_GROUPS.setdefault(mb + nmap - 1, []).append((vy, mb, nmap, vxmin))

            # ---- guide phase: per-map pipeline (Pool sub -> ACT sq ->
            #      DVE c-reduce -> ACT exp w/ per-map spatial bias) ----
            for m, (uy, ux) in enumerate(UPLUS):
                sl = m % NSLOT
                in0 = sb(G4, 0, MR, uy * CG * GX + 3 + ux, [[GX, CG], [1, MJ]])
                in1 = sb(G4, 0, MR, 3, [[GX, CG], [1, MJ]])
                d3 = sb(D3R, 0, MR, sl * CG * MJ, [[MJ, CG], [1, MJ]])
                sub_eng = v if m < 17 else g
                sub_eng.tensor_tensor(out=d3, in0=in0, in1=in1, op=ALU.subtract)
                s.activation(out=sb(SQR, 0, MR, sl * CG * MJ, [[1, CG * MJ]]),
                             in_=sb(D3R, 0, MR, sl * CG * MJ, [[1, CG * MJ]]),
                             func=ACTF.Square)
                _red = v.tensor_reduce(out=sb(SRR, 0, MR, sl * MJ, [[1, MJ]]),
                                in_=sb(SQR, 0, MR, sl * CG * MJ,
                                       [[1, MJ], [MJ, CG]]),
                                axis=mybir.AxisListType.X, op=ALU.add)
                if m == 24:
                    last_reduce = _red
                s.activation(out=sb(K25, 0, MR, m * MS, [[1, MJ]]),
                             in_=sb(SRR, 0, MR, sl * MJ, [[1, MJ]]),
                             func=ACTF.Exp, scale=-0.5,
                             bias=BROWT[0:MR, m:m + 1])
                # kA remap DMAs for batches completed by this map
                for (m0c, nmap) in AL_CHUNKS.get(m, ()):
                    for wh in range(WH):
                        dst = sb(KA, wh * 64, 64, m0c * XC, [[XC, nmap], [1, XC]])
                        srcc = sb(K25, 3, 64, m0c * MS + wh * XC + 3,
                                  [[MS, nmap], [1, XC]])
                        s.dma_start(out=dst, in_=srcc)
                for (vy, mb, nmap, vxmin) in SH_GROUPS.get(m, ()):
                    for wh in range(WH):
                        dst = sb(KA, wh * 64, 64, (24 + mb) * XC,
                                 [[XC, nmap], [1, XC]])
                        srcc = sb(K25, 3 - vy, 64,
                                  mb * MS + wh * XC + 3 - vxmin,
                                  [[MS - 1, nmap], [1, XC]])
                        s.dma_start(out=dst, in_=srcc)

            # ---- norm (Pool STT accumulate chain) + reciprocal (DVE) ----
            g.memset(NORM[:, :], 0.0)
            for u in range(49):
                g.tensor_tensor(out=NORM[:, :],
                                in0=sb(KA, 0, 128, u * XC, [[1, XC]]),
                                in1=NORM[:, :], op=ALU.add)
            v.reciprocal(out=RCP[:, :], in_=NORM[:, :])

            # ---- odd-parity (x-1) copy, per dr-slice on ACT (4) + Pool (3)
            #      from INB7 in SBUF (keeps DMA engines free) ----
            for di, dr in enumerate(DR_ORDER):
                dst = sb(INB7O, 0, 128, dr * XW + 1, [[7 * XW, C], [1, XW - 1]])
                srcc = sb(INB7, 0, 128, dr * XW, [[7 * XW, C], [1, XW - 1]])
                if di % 2 == 0:
                    s.copy(out=dst, in_=srcc)
                else:
                    g.tensor_copy(dst, srcc)

            # ---- apply: 49 offsets; even-parity first, by map readiness ----
            evens = sorted((e for e in ALL_U if (3 + e[1]) % 2 == 0),
                           key=lambda e: e[2])
            odds = sorted((e for e in ALL_U if (3 + e[1]) % 2 == 1),
                          key=lambda e: e[2])
            order = evens + odds
            first = True
            for oi, ent in enumerate(order):
                uy, ux, m, shifted = ent
                ui = ka_slot[(uy, ux)]
                off = (uy + 3) * XW + 3 + ux
                if (3 + ux) % 2 == 0:
                    in0 = sb(INB7, 0, 128, off, [[7 * XW, C], [1, XC]])
                else:
                    in0 = sb(INB7O, 0, 128, off + 1, [[7 * XW, C], [1, XC]])
                in1 = sb(KA, 0, 128, ui * XC, [[0, C], [1, XC]])
                P = PBUFS[oi % 4]
                _tt = v.tensor_tensor(out=sb(P, 0, 128, 0, [[XC, C], [1, XC]]),
                                      in0=in0, in1=in1, op=ALU.mult)
                if oi < 4:
                    from concourse.tile import add_dep_helper
                    add_dep_helper(_tt.ins, last_reduce.ins, sync=False,
                                   reason="hold applies until guide reduces done (PE warmup)")
                last = oi == len(order) - 1
                for bk in range(8):
                    t.matmul(ACC[:, bk * 512:(bk + 1) * 512], IDENT[:, :],
                             P[:, bk * 512:(bk + 1) * 512], start=first, stop=last)
                first = False

            # ---- finish: out = acc * rcp (bcast over c), 4 chunks of 8 ch ----
            for ch in range(4):
                obuf = (ch % 2) * 8 * XC
                a_sl = ACC[:, ch * 1024:(ch + 1) * 1024]
                a_ap = AP(a_sl.tensor, a_sl.offset, [a_sl.ap[0], [XC, 8], [1, XC]])
                r_ap = sb(RCP, 0, 128, 0, [[0, 8], [1, XC]])
                o_ap = sb(OUTC, 0, 128, obuf, [[XC, 8], [1, XC]])
                v.tensor_tensor(out=o_ap, in0=a_ap, in1=r_ap, op=ALU.mult)
                for wh in range(WH):
                    srcc = sb(OUTC, wh * 64, 64, obuf, [[XC, 8], [1, XC]])
                    dst = dr_ap(out_d, ch * 8 * RB * W + wh * XC,
                                [[W, 64], [RB * W, 8], [1, XC]])
                    sync.dma_start(out=dst, in_=srcc)

    if legalize:
        _legalize_waits(nc)
    return nc


def _legalize_waits(nc):
    """walrus codegen allows 1 sem-wait on DMA instructions (2 elsewhere);
    Tile can emit more. Move excess waits onto InstEventSemaphore nops
    inserted just before, on the same engine (sequencer stalls, then issues)."""
    import concourse.mybir as mybir

    ctr = [0]
    for bb in nc.main_func.blocks:
        out = []
        changed = False
        for ins in bb.instructions:
            cap = 1
            si = ins.sync_info
            waits = list(si.on_wait) if si is not None else []
            if len(waits) > cap:
                keep = waits[:cap]
                extra = waits[cap:]
                while extra:
                    chunk, extra = extra[:1], extra[1:]
                    e = mybir.InstEventSemaphore(
                        name=f"wsplit-{ctr[0]}", ins=[], outs=[])
                    ctr[0] += 1
                    e.engine = ins.engine
                    e.sync_info = mybir.SyncInfo(on_wait=chunk, on_update=[])
                    out.append(e)
                ins.sync_info = mybir.SyncInfo(on_wait=keep, on_update=list(si.on_update))
                changed = True
            out.append(ins)
        if changed:
            bb.instructions = out
    return nc


def _host_prep(input, input_for_kernel, sigma_for_kernel):
    inp = np.asarray(input, dtype=np.float32)
    gui = np.asarray(input_for_kernel, dtype=np.float32)
    sig = np.float32(np.asarray(sigma_for_kernel).reshape(()))

    # pad rows/cols by 6 each side (covers all slice windows with zeros)
    gp = np.zeros((B, CG, H + 12, W + 12), dtype=np.float32)
    gp[:, :, 6:6 + H, 6:6 + W] = gui
    ip = np.zeros((B, C, H + 12, W + 12), dtype=np.float32)
    ip[:, :, 6:6 + H, 6:6 + W] = inp

    rr = np.array([[float(uy * uy + ux * ux) for (uy, ux) in UPLUS]],
                  dtype=np.float32)
    ident = np.eye(128, dtype=np.float16)
    sig_arr = np.array([[sig]], dtype=np.float32)

    in_maps = []
    for core in range(NCORES):
        b, hb = divmod(core, NB)
        r0 = hb * RB
        # guide rows r0-3..r0+66, cols -6..258 -> gp[rows 6+r0-3 .., cols 0:265]
        gs = gp[b, :, 3 + r0: 3 + r0 + GR, 0:GX]
        # input rows r0-3..r0+66, cols -3..258 -> ip cols 3:3+262
        is_ = ip[b, :, 3 + r0: 3 + r0 + GR, 3:3 + IX]
        in_maps.append({
            "guide": np.ascontiguousarray(gs),
            "inp": np.ascontiguousarray(is_),
            "rr25": rr,
            "sigma": sig_arr,
            "ident": ident,
        })
    return in_maps


def kernel(input, input_for_kernel, sigma_for_kernel):
    global _COMPILED
    from concourse.bass_utils import run_bass_kernel_spmd

    if _COMPILED is None:
        _COMPILED = _build_nc()
    nc = _COMPILED

    in_maps = _host_prep(input, input_for_kernel, sigma_for_kernel)
    res = run_bass_kernel_spmd(nc, in_maps, core_ids=list(range(NCORES)))
    out = np.zeros((B, C, H, W), dtype=np.float32)
    for core in range(NCORES):
        b, hb = divmod(core, NB)
        out[b, :, hb * RB:(hb + 1) * RB, :] = res.results[core]["out"]
    return out


# revision 35
# speedup vs baseline: 7.8059x; 7.8059x over previous
"""Bilateral filter (K=7, guide channels=3) Trainium2 Bass kernel.

Contract: kernel(**inputs) takes FULL unsharded numpy inputs
(input [2,32,256,256] f32, input_for_kernel [2,3,256,256] f32,
sigma_for_kernel scalar f32) and returns the full output [2,32,256,256] f32.
Shards internally over 8 NeuronCores: (batch=2) x (4 h-blocks of 64 rows).

Math (identical to the reference up to fp rounding; the spatial-gaussian
normalization cancels in ker/norm):
  m_u[p]   = exp(-0.5*sum_c (g[c,p+u]-g[c,p])^2) * exp(-0.5*(uy^2+ux^2)/s^2)
  out[c,p] = sum_u m_u[p]*in[c,p+u] / sum_u m_u[p]        (zero padding)

Symmetry m_{-u}[p] = m_u[p-u] -> only 25 maps computed on an extended grid,
all 49 aligned maps then materialized by DMA remap (DMA can cross partitions;
compute engines cannot, so every row shift lives in a free dim).
"""

import numpy as np

B, C, H, W = 2, 32, 256, 256
CG = 3
R = 3                      # K//2
NB = 4                     # h-blocks per batch
RB = H // NB               # 64 out rows per core
NCORES = 8

GR = RB + 2 * R            # 70 rows   (out rows -3..66)
GX = W + 4 * R             # 268 guide cols (-6..261)
IX = W + 2 * R             # 262 input cols (-3..258)
MR = RB + R                # 67 map rows (-3..63)
MJ = W + 2 * R             # 262 map cols (-3..258)
MS = W + 2 * R             # 262 per-map stride in K25 (even)
WH = 2                     # w halves in apply layout
XC = W // WH               # 128
XW = XC + 2 * R            # 134 per-half x window

UPLUS = [(0, ux) for ux in range(0, R + 1)] + [
    (uy, ux) for uy in range(1, R + 1) for ux in range(-R, R + 1)
]
ALL_U = []
for uy in range(-R, R + 1):
    for ux in range(-R, R + 1):
        if (uy, ux) in UPLUS:
            ALL_U.append((uy, ux, UPLUS.index((uy, ux)), False))
        else:
            ALL_U.append((uy, ux, UPLUS.index((-uy, -ux)), True))

_COMPILED = None


def _build_nc(legalize=True):
    import concourse.bass as bass
    import concourse.mybir as mybir
    from concourse.bass import AP
    from concourse.tile import TileContext

    fp32 = mybir.dt.float32
    fp16 = mybir.dt.float16
    ALU = mybir.AluOpType
    ACTF = mybir.ActivationFunctionType

    nc = bass.Bass()

    guide_d = nc.declare_dram_parameter("guide", [CG, GR, GX], fp32, isOutput=False)
    inp_d = nc.declare_dram_parameter("inp", [C, GR, IX], fp32, isOutput=False)
    rr_d = nc.declare_dram_parameter("rr25", [1, 25], fp32, isOutput=False)
    sig_d = nc.declare_dram_parameter("sigma", [1, 1], fp32, isOutput=False)
    ident_d = nc.declare_dram_parameter("ident", [128, 128], fp16, isOutput=False)
    out_d = nc.declare_dram_parameter("out", [C, RB, W], fp32, isOutput=True)

    def sb(t, p0, pn, off, dims):
        """AP into sbuf/psum tensor: partitions [p0, p0+pn), free offset+dims."""
        sl = t[p0:p0 + pn]
        return AP(sl.tensor, sl.offset + off, [sl.ap[0], *dims])

    def dr_ap(d, off, dims):
        full = d[:]
        return AP(full.tensor, full.offset + off, dims)

    from contextlib import ExitStack

    with TileContext(nc) as tc, ExitStack() as es:
        def mk(name, shape, dt):
            return es.enter_context(nc.sbuf_tensor(name, shape, dt))

        # aliasing region: guide-phase scratch (G4/D3B/SREDG/INH) is dead by
        # the time INB7O (odd-parity input copy) is written; OverlapTracker
        # fences reads->writes by byte range.
        _base0 = ((nc.sbuf_base + 31) // 32) * 32
        _ARENA_BYTES = 190 * 1024
        es.enter_context(nc.sbuf_tensor("ARENA", [128, _ARENA_BYTES], mybir.dt.uint8))
        _off = [_base0]
        def at(name, shape, dt, offset=None):
            if offset is None:
                offset = _off[0]
            import functools, operator
            sz = functools.reduce(operator.mul, shape[1:]) * mybir.dt.size(dt)
            h = nc.alloc_sbuf_tensor_at(name, shape, dt, offset=offset, align_bytes=32)
            _off[0] = max(_off[0], offset + ((sz + 31) // 32) * 32)
            return h

        INB7 = at("INB7", [128, C * 7 * XW], fp16)          # (c,dr,x)
        _arena = _off[0]
        INB7O = at("INB7O", [128, C * 7 * XW], fp16, _arena)
        # guide-phase scratch aliases INB7O's bytes (dead before INB7O written)
        def _al(n):
            return ((n + 31) // 32) * 32
        NSLOT = 6
        _g4b = _al(4 * CG * GX * 4)
        _d3b = _al(NSLOT * CG * MJ * 4)
        _sqb = _al(NSLOT * CG * MJ * 4)
        _srb = _al(NSLOT * MJ * 4)
        G4 = at("G4", [128, 4 * CG * GX], fp32, _arena)
        D3R = at("D3R", [128, NSLOT * CG * MJ], fp32, _arena + _g4b)
        SQR = at("SQR", [128, NSLOT * CG * MJ], fp32, _arena + _g4b + _d3b)
        SRR = at("SRR", [128, NSLOT * MJ], fp32, _arena + _g4b + _d3b + _sqb)
        assert _g4b + _d3b + _sqb + _srb <= C * 7 * XW * 2, (
            _g4b + _d3b + _sqb + _srb, C * 7 * XW * 2)
        _off[0] = _arena + ((C * 7 * XW * 2 + 31) // 32) * 32
        K25 = at("K25", [128, 25 * MS], fp16)               # (m,j)
        KA = at("KA", [128, 49 * XC], fp16)                 # (u,x)
        NORM = at("NORM", [128, XC], fp32)
        RCP = at("RCP", [128, XC], fp32)
        BROWT = at("BROWT", [128, 25], fp32)
        IDENT = at("IDENT", [128, 128], fp16)
        PBUFS = [at(f"P{i}T", [128, 4096], fp16) for i in range(4)]
        OUTC = at("OUTC", [128, 2 * 8 * XC], fp32)
        BROW = at("BROW", [1, 25], fp32)
        SIG = at("SIG", [1, 1], fp32)
        SIG2 = at("SIG2", [1, 1], fp32)
        IS2 = at("IS2", [1, 1], fp32)
        RR = at("RR", [1, 25], fp32)
        assert _off[0] <= _base0 + _ARENA_BYTES, (_off[0], _base0)
        ACC = es.enter_context(nc.psum_tensor("ACC", [128, 4096], fp32))
        if True:
            v, s, g, t, sync = nc.vector, nc.scalar, nc.gpsimd, nc.tensor, nc.sync

            # ---- guide load first (gates the first subs) ----
            for dy in range(4):
                dst = sb(G4, 0, MR, dy * (CG * GX), [[GX, CG], [1, GX]])
                src = dr_ap(guide_d, dy * GX, [[GX, MR], [GR * GX, CG], [1, GX]])
                sync.dma_start(out=dst, in_=src)

            # ---- constants ----
            sync.dma_start(out=RR[:], in_=rr_d[:])
            sync.dma_start(out=SIG[:], in_=sig_d[:])
            sync.dma_start(out=IDENT[:], in_=ident_d[:])
            v.tensor_tensor(out=SIG2[:], in0=SIG[:], in1=SIG[:], op=ALU.mult)
            v.reciprocal(out=IS2[:], in_=SIG2[:])
            v.tensor_scalar(out=BROW[:], in0=RR[:], scalar1=IS2[0:1, 0:1],
                            scalar2=-0.5, op0=ALU.mult, op1=ALU.mult)
            # broadcast partition 0 -> all via DRAM round-trip (0-step read)
            wscr = nc.dram_tensor("wscr", [1, 25], fp32)
            sync.dma_start(out=wscr[:], in_=BROW[:])
            sync.dma_start(out=BROWT[:],
                           in_=dr_ap(wscr, 0, [[0, 128], [1, 25]]))

            # ---- stage fp16 input in DRAM (software-DGE cast DMA), then
            #      row-replicated loads straight from DRAM (overlapping reads).
            #      1 leading pad element so the x-1 (odd-parity) reads stay
            #      in bounds. ----
            INHD = nc.dram_tensor("INHD", [1, C * GR * IX + 2], fp16)
            g.dma_start(out=dr_ap(INHD, 1, [[1, C * GR * IX]]),
                        in_=dr_ap(inp_d, 0, [[1, C * GR * IX]]))
            # dr order matches apply-map order (m=0 is uy=0 -> dr3 first)
            DR_ORDER = [3, 4, 2, 5, 1, 6, 0]
            for dr in DR_ORDER:
                for wh in range(WH):
                    dst = sb(INB7, wh * 64, 64, dr * XW, [[7 * XW, C], [1, XW]])
                    src = dr_ap(INHD, 1 + dr * IX + wh * XC,
                                [[IX, 64], [GR * IX, C], [1, XW]])
                    sync.dma_start(out=dst, in_=src)

            # kA slot order: aligned maps at slot=m, shifted at 24+m
            ka_slot = {}
            for (uy, ux, m, shifted) in ALL_U:
                ka_slot[(uy, ux)] = m if not shifted else 24 + m
            # batch tables keyed by the last map index that completes them
            AL_CHUNKS = {}
            CH = 4
            for m0c in range(0, 25, CH):
                nmap = min(CH, 25 - m0c)
                AL_CHUNKS.setdefault(m0c + nmap - 1, []).append((m0c, nmap))
            SH_GROUPS = {}
            _vy_base = {0: 1, 1: 4, 2: 11, 3: 18}
            for (vy, mb, nmap) in [(0, 1, 3), (1, 4, 4), (1, 8, 3), (2, 11, 4),
                                   (2, 15, 3), (3, 18, 4), (3, 22, 3)]:
                vxmin = ((-3 if vy >= 1 else 1) + (mb - _vy_base[vy]))
                SH_GROUPS.setdefault(mb + nmap - 1, []).append((vy, mb, nmap, vxmin))

            # ---- guide phase: per-map pipeline (Pool sub -> ACT sq ->
            #      DVE c-reduce -> ACT exp w/ per-map spatial bias) ----
            for m, (uy, ux) in enumerate(UPLUS):
                sl = m % NSLOT
                in0 = sb(G4, 0, MR, uy * CG * GX + 3 + ux, [[GX, CG], [1, MJ]])
                in1 = sb(G4, 0, MR, 3, [[GX, CG], [1, MJ]])
                d3 = sb(D3R, 0, MR, sl * CG * MJ, [[MJ, CG], [1, MJ]])
                sub_eng = v if m < 9 else g
                sub_eng.tensor_tensor(out=d3, in0=in0, in1=in1, op=ALU.subtract)
                s.activation(out=sb(SQR, 0, MR, sl * CG * MJ, [[1, CG * MJ]]),
                             in_=sb(D3R, 0, MR, sl * CG * MJ, [[1, CG * MJ]]),
                             func=ACTF.Square)
                _red = v.tensor_reduce(out=sb(SRR, 0, MR, sl * MJ, [[1, MJ]]),
                                in_=sb(SQR, 0, MR, sl * CG * MJ,
                                       [[1, MJ], [MJ, CG]]),
                                axis=mybir.AxisListType.X, op=ALU.add)
                if m == 24:
                    last_reduce = _red
                s.activation(out=sb(K25, 0, MR, m * MS, [[1, MJ]]),
                             in_=sb(SRR, 0, MR, sl * MJ, [[1, MJ]]),
                             func=ACTF.Exp, scale=-0.5,
                             bias=BROWT[0:MR, m:m + 1])
                # PE warm-keeper: dummy matmuls spread through the guide
                # (clobbered by the first start=True accumulation)
                from concourse.tile import add_dep_helper
                for _w in range(2):
                    _mm = t.matmul(ACC[:, 3584:4096], IDENT[:, :],
                                   sb(K25, 0, 128, 0, [[1, 512]]),
                                   start=True, stop=True, skip_group_check=True)
                    add_dep_helper(_mm.ins, _red.ins, sync=False,
                                   reason="spread PE warmup")
                # kA remap DMAs for batches completed by this map
                for (m0c, nmap) in AL_CHUNKS.get(m, ()):
                    for wh in range(WH):
                        dst = sb(KA, wh * 64, 64, m0c * XC, [[XC, nmap], [1, XC]])
                        srcc = sb(K25, 3, 64, m0c * MS + wh * XC + 3,
                                  [[MS, nmap], [1, XC]])
                        s.dma_start(out=dst, in_=srcc)
                for (vy, mb, nmap, vxmin) in SH_GROUPS.get(m, ()):
                    for wh in range(WH):
                        dst = sb(KA, wh * 64, 64, (24 + mb) * XC,
                                 [[XC, nmap], [1, XC]])
                        srcc = sb(K25, 3 - vy, 64,
                                  mb * MS + wh * XC + 3 - vxmin,
                                  [[MS - 1, nmap], [1, XC]])
                        s.dma_start(out=dst, in_=srcc)

            # ---- norm (Pool STT accumulate chain) + reciprocal (DVE) ----
            g.memset(NORM[:, :], 0.0)
            for u in range(49):
                g.tensor_tensor(out=NORM[:, :],
                                in0=sb(KA, 0, 128, u * XC, [[1, XC]]),
                                in1=NORM[:, :], op=ALU.add)
            v.reciprocal(out=RCP[:, :], in_=NORM[:, :])

            # ---- odd-parity (x-1) copy, per dr-slice on ACT (4) + Pool (3)
            #      from INB7 in SBUF (keeps DMA engines free) ----
            for di, dr in enumerate(DR_ORDER):
                dst = sb(INB7O, 0, 128, dr * XW + 1, [[7 * XW, C], [1, XW - 1]])
                srcc = sb(INB7, 0, 128, dr * XW, [[7 * XW, C], [1, XW - 1]])
                if di % 2 == 0:
                    s.copy(out=dst, in_=srcc)
                else:
                    g.tensor_copy(dst, srcc)

            # ---- apply: 49 offsets; even-parity first, by map readiness ----
            evens = sorted((e for e in ALL_U if (3 + e[1]) % 2 == 0),
                           key=lambda e: e[2])
            odds = sorted((e for e in ALL_U if (3 + e[1]) % 2 == 1),
                          key=lambda e: e[2])
            order = evens + odds
            first = True
            for oi, ent in enumerate(order):
                uy, ux, m, shifted = ent
                ui = ka_slot[(uy, ux)]
                off = (uy + 3) * XW + 3 + ux
                if (3 + ux) % 2 == 0:
                    in0 = sb(INB7, 0, 128, off, [[7 * XW, C], [1, XC]])
                else:
                    in0 = sb(INB7O, 0, 128, off + 1, [[7 * XW, C], [1, XC]])
                in1 = sb(KA, 0, 128, ui * XC, [[0, C], [1, XC]])
                P = PBUFS[oi % 4]
                v.tensor_tensor(out=sb(P, 0, 128, 0, [[XC, C], [1, XC]]),
                                in0=in0, in1=in1, op=ALU.mult)
                last = oi == len(order) - 1
                for bk in range(8):
                    t.matmul(ACC[:, bk * 512:(bk + 1) * 512], IDENT[:, :],
                             P[:, bk * 512:(bk + 1) * 512], start=first, stop=last)
                first = False

            # ---- finish: out = acc * rcp (bcast over c), 4 chunks of 8 ch ----
            for ch in range(4):
                obuf = (ch % 2) * 8 * XC
                a_sl = ACC[:, ch * 1024:(ch + 1) * 1024]
                a_ap = AP(a_sl.tensor, a_sl.offset, [a_sl.ap[0], [XC, 8], [1, XC]])
                r_ap = sb(RCP, 0, 128, 0, [[0, 8], [1, XC]])
                o_ap = sb(OUTC, 0, 128, obuf, [[XC, 8], [1, XC]])
                v.tensor_tensor(out=o_ap, in0=a_ap, in1=r_ap, op=ALU.mult)
                for wh in range(WH):
                    srcc = sb(OUTC, wh * 64, 64, obuf, [[XC, 8], [1, XC]])
                    dst = dr_ap(out_d, ch * 8 * RB * W + wh * XC,
                                [[W, 64], [RB * W, 8], [1, XC]])
                    sync.dma_start(out=dst, in_=srcc)

    if legalize:
        _legalize_waits(nc)
    return nc


def _legalize_waits(nc):
    """walrus codegen allows 1 sem-wait on DMA instructions (2 elsewhere);
    Tile can emit more. Move excess waits onto InstEventSemaphore nops
    inserted just before, on the same engine (sequencer stalls, then issues)."""
    import concourse.mybir as mybir

    ctr = [0]
    for bb in nc.main_func.blocks:
        out = []
        changed = False
        for ins in bb.instructions:
            cap = 1
            si = ins.sync_info
            waits = list(si.on_wait) if si is not None else []
            if len(waits) > cap:
                keep = waits[:cap]
                extra = waits[cap:]
                while extra:
                    chunk, extra = extra[:1], extra[1:]
                    e = mybir.InstEventSemaphore(
                        name=f"wsplit-{ctr[0]}", ins=[], outs=[])
                    ctr[0] += 1
                    e.engine = ins.engine
                    e.sync_info = mybir.SyncInfo(on_wait=chunk, on_update=[])
                    out.append(e)
                ins.sync_info = mybir.SyncInfo(on_wait=keep, on_update=list(si.on_update))
                changed = True
            out.append(ins)
        if changed:
            bb.instructions = out
    return nc


def _host_prep(input, input_for_kernel, sigma_for_kernel):
    inp = np.asarray(input, dtype=np.float32)
    gui = np.asarray(input_for_kernel, dtype=np.float32)
    sig = np.float32(np.asarray(sigma_for_kernel).reshape(()))

    # pad rows/cols by 6 each side (covers all slice windows with zeros)
    gp = np.zeros((B, CG, H + 12, W + 12), dtype=np.float32)
    gp[:, :, 6:6 + H, 6:6 + W] = gui
    ip = np.zeros((B, C, H + 12, W + 12), dtype=np.float32)
    ip[:, :, 6:6 + H, 6:6 + W] = inp

    rr = np.array([[float(uy * uy + ux * ux) for (uy, ux) in UPLUS]],
                  dtype=np.float32)
    ident = np.eye(128, dtype=np.float16)
    sig_arr = np.array([[sig]], dtype=np.float32)

    in_maps = []
    for core in range(NCORES):
        b, hb = divmod(core, NB)
        r0 = hb * RB
        # guide rows r0-3..r0+66, cols -6..258 -> gp[rows 6+r0-3 .., cols 0:265]
        gs = gp[b, :, 3 + r0: 3 + r0 + GR, 0:GX]
        # input rows r0-3..r0+66, cols -3..258 -> ip cols 3:3+262
        is_ = ip[b, :, 3 + r0: 3 + r0 + GR, 3:3 + IX]
        in_maps.append({
            "guide": np.ascontiguousarray(gs),
            "inp": np.ascontiguousarray(is_),
            "rr25": rr,
            "sigma": sig_arr,
            "ident": ident,
        })
    return in_maps


def kernel(input, input_for_kernel, sigma_for_kernel):
    global _COMPILED
    from concourse.bass_utils import run_bass_kernel_spmd

    if _COMPILED is None:
        _COMPILED = _build_nc()
    nc = _COMPILED

    in_maps = _host_prep(input, input_for_kernel, sigma_for_kernel)
    res = run_bass_kernel_spmd(nc, in_maps, core_ids=list(range(NCORES)))
    out = np.zeros((B, C, H, W), dtype=np.float32)
    for core in range(NCORES):
        b, hb = divmod(core, NB)
        out[b, :, hb * RB:(hb + 1) * RB, :] = res.results[core]["out"]
    return out
